# revision 1
# baseline (speedup 1.0000x reference)
"""KAN layer (B-spline + SiLU) Trainium2 kernel.

Math: y[b,k] = scale * sum_i( silu(x[b,i])*W[i,k]
                              + sum_j basis_j(x[b,i]) * C[i,k,j] )

With a uniform grid (12 knots, spacing h, first knot g0), the cubic
B-spline basis is  basis_j(x) = B3(u - j),  u = (x - g0)/h, u in [0,11),
and B3 expands into truncated relu-cubes:
  B3(u-j) = sum_m (-1)^m C(4,m)/6 * relu(u-j-m)^3
On the masked domain u < 11 only shifts s=0..10 survive, so
  y = [silu(x), r_0..r_10] @ Wfull,   r_s = relu(u-s)^3
a [B, 12*128] @ [12*128, 128] matmul. Features are computed on DVE with
custom fused ops; the matmul runs in float32r (full PE rate at N=512).

Sharding: data-parallel over batch, 1024 rows per core on 8 cores.
"""

import math
import os
import sys

import numpy as np

if "/opt/trn_rl_repo" not in sys.path:
    sys.path.insert(0, "/opt/trn_rl_repo")

import concourse.bass as bass
import concourse.mybir as mybir
from concourse import bacc
from concourse.tile import TileContext

B_TOTAL = 8192
IN_DIM = 128
OUT_DIM = 128
N_CORES = 8
B_CORE = B_TOTAL // N_CORES  # 1024
NB = 8    # num basis
NS = 11   # truncated-cube shifts s=0..10
NCHUNK = NS + 1  # + silu chunk

F32 = mybir.dt.float32
F32R = mybir.dt.float32r

# ---------------------------------------------------------------- custom DVE ops


def _register_ops():
    from concourse.dve_ops import (
        _CUSTOM_DVE_ROW_BASE,
        _SUB_OPCODE_FOR_NAME,
        CUSTOM_DVE_SPECS,
        OPS,
        DveOp,
    )
    from concourse.dve_spec import C0, C1, C2, Spec, Src0, lower, relu, sq
    from concourse.dve_uop import DveOpSpec

    def reg(name, spec):
        for op in OPS:
            if op.name == name:
                return op
        row = _CUSTOM_DVE_ROW_BASE + len(OPS)
        assert row < 0x20
        _SUB_OPCODE_FOR_NAME[name] = row
        shas = {}
        for ver in ("v3", "v4"):
            s = DveOpSpec(name=name, opcode=row, uops=lower(spec, ver=ver),
                          rd1_en=False)
            shas[ver] = s.sha(ver)
        op = DveOp(name, spec, subdim=False, uops_sha=shas)
        OPS.append(op)
        CUSTOM_DVE_SPECS[name] = spec
        return op

    # u0 = C0*(x - C2*(x >= C1)) : scaled, masked u (minus the constant part)
    mask_affine = Spec(
        body=(Src0 - C2 * (Src0 >= C1)) * C0,
        reference=lambda in0, in1, s0, s1, imm2: (
            (in0 - imm2 * (in0 >= s1)) * s0
        ).astype(np.float32),
    )
    # r = relu(t)^2 * t  with t = u0 + C0  ( = relu(t)^3 )
    _t = Src0 + C0
    relu_cube = Spec(
        body=sq(relu(_t)) * _t,
        reference=lambda in0, in1, s0, s1, imm2: (
            np.maximum(in0 + s0, 0.0) ** 2 * (in0 + s0)
        ).astype(np.float32),
    )
    return reg("ANT_KAN_MASK_AFFINE", mask_affine), reg("ANT_KAN_RELU_CUBE", relu_cube)


OP_MASK_AFFINE, OP_RELU_CUBE = _register_ops()

# ---------------------------------------------------------------- device kernel

_NC_CACHE = {}


def _build_nc():
    if "nc" in _NC_CACHE:
        return _NC_CACHE["nc"]
    nc = bacc.Bacc("TRN2", target_bir_lowering=False)
    xT = nc.dram_tensor("xT", [IN_DIM, B_CORE], F32, kind="ExternalInput")
    # weights pre-arranged on host as [i, chunk, k] so the DMA is contiguous
    wf = nc.dram_tensor("wf", [IN_DIM, NCHUNK, OUT_DIM], F32, kind="ExternalInput")
    yT = nc.dram_tensor("yT", [OUT_DIM, B_CORE], F32, kind="ExternalOutput")

    NHALF = B_CORE // 512

    with TileContext(nc) as tc:
        with (
            tc.tile_pool(name="wpool", bufs=1) as wpool,
            tc.tile_pool(name="dpool", bufs=1) as dpool,
            tc.tile_pool(name="ppool", bufs=2, space="PSUM") as ppool,
        ):
            wt = wpool.tile([IN_DIM, NCHUNK, OUT_DIM], F32, tag="wt")
            nc.sync.dma_start(out=wt[:], in_=wf[:])

            xt = dpool.tile([IN_DIM, B_CORE], F32, tag="xt")
            nc.sync.dma_start(out=xt[:], in_=xT[:])

            feat = dpool.tile([IN_DIM, NCHUNK, B_CORE], F32, tag="feat")
            u0 = dpool.tile([IN_DIM, B_CORE], F32, tag="u0")

            # silu chunk (last in accumulation order, first issued on ACT)
            nc.scalar.activation(feat[:, NS, :], xt[:],
                                 mybir.ActivationFunctionType.Silu)

            # u0 = 2.5*(x - 100*(x >= 2.2))   [grid-derived consts patched in]
            nc.vector._custom_dve(
                OP_MASK_AFFINE, out=u0[:], in0=xt[:],
                s0=_NC_CACHE["inv_h"], s1=_NC_CACHE["xmax"], imm2=100.0,
            )
            # r_s = relu(u0 + (c0 - s))^3
            for s in range(NS):
                nc.vector._custom_dve(
                    OP_RELU_CUBE, out=feat[:, s, :], in0=u0[:],
                    s0=_NC_CACHE["u_off"] - float(s),
                )

            yt = dpool.tile([OUT_DIM, B_CORE], F32, tag="yt")
            for h in range(NHALF):
                ps = ppool.tile([OUT_DIM, 512], F32, tag=f"ps{h}")
                for j in range(NCHUNK):
                    nc.tensor.matmul(
                        ps[:],
                        lhsT=wt[:, j, :],
                        rhs=feat[:, j, h * 512:(h + 1) * 512],
                        start=(j == 0),
                        stop=(j == NCHUNK - 1),
                    )
                nc.scalar.activation(yt[:, h * 512:(h + 1) * 512], ps[:],
                                     mybir.ActivationFunctionType.Copy)
                nc.sync.dma_start(out=yT[:, h * 512:(h + 1) * 512],
                                  in_=yt[:, h * 512:(h + 1) * 512])

    nc.finalize()
    _NC_CACHE["nc"] = nc
    return nc


# ---------------------------------------------------------------- host wrapper


def _build_weights(grid, spline_coeff, base_weight, scale):
    g0 = float(grid[0, 0])
    h = float(grid[0, 1] - grid[0, 0])
    sc = float(scale.reshape(-1)[0])
    # D[j, s]: coefficient of relu(u-s)^3 in B3(u-j), s <= NS-1
    D = np.zeros((NB, NS), dtype=np.float64)
    for j in range(NB):
        for m in range(5):
            s = j + m
            if s < NS:
                D[j, s] = (-1.0) ** m * math.comb(4, m) / 6.0
    C2 = np.einsum("ikj,js->iks", spline_coeff.astype(np.float64), D)
    # [i, chunk, k]: chunks 0..NS-1 are cube shifts, chunk NS is silu/base
    wfull = np.empty((IN_DIM, NCHUNK, OUT_DIM), dtype=np.float32)
    wfull[:, :NS, :] = (C2.transpose(0, 2, 1) * sc).astype(np.float32)
    wfull[:, NS, :] = (base_weight.astype(np.float64) * sc).astype(np.float32)
    return wfull, g0, h


def kernel(x, grid, spline_coeff, base_weight, scale):
    from concourse.bass_utils import run_bass_kernel_spmd

    wfull, g0, h = _build_weights(grid, spline_coeff, base_weight, scale)
    inv_h = 1.0 / h                      # 2.5
    xmax = g0 + 11.0 * h                 # 2.2  (mask threshold: u < 11)
    # u = (x - g0)/h = x/h - g0/h ; u0 = x/h (masked); cube shift c0 = -g0/h - s
    u_off = -g0 / h                      # 5.5

    _NC_CACHE.setdefault("inv_h", float(inv_h))
    _NC_CACHE.setdefault("xmax", float(xmax))
    _NC_CACHE.setdefault("u_off", float(u_off))

    nc = _build_nc()

    xT = np.ascontiguousarray(x.astype(np.float32).T)  # [128, 8192]
    in_maps = []
    for c in range(N_CORES):
        in_maps.append({
            "xT": np.ascontiguousarray(xT[:, c * B_CORE:(c + 1) * B_CORE]),
            "wf": wfull,
        })

    res = run_bass_kernel_spmd(nc, in_maps, core_ids=list(range(N_CORES)))
    outs = res.results
    yT = np.concatenate([outs[c]["yT"] for c in range(N_CORES)], axis=1)
    return np.ascontiguousarray(yT.T)


if __name__ == "__main__":
    rng = np.random.default_rng(0)
    x = rng.standard_normal((B_TOTAL, IN_DIM)).astype(np.float32)
    g = np.linspace(-1, 1, 6)
    hh = 0.4
    for _ in range(3):
        g = np.concatenate([[g[0] - hh], g, [g[-1] + hh]])
    grid = np.broadcast_to(g.astype(np.float32), (IN_DIM, 12)).copy()
    C = rng.standard_normal((IN_DIM, OUT_DIM, NB)).astype(np.float32)
    W = rng.standard_normal((IN_DIM, OUT_DIM)).astype(np.float32)
    s = np.ones((1,), np.float32)
    y = kernel(x, grid, C, W, s)
    print(y.shape, y.dtype, np.abs(y).max())



# revision 3
# speedup vs baseline: 1.2863x; 1.2863x over previous
"""KAN layer (B-spline + SiLU) Trainium2 kernel, v2.

Math: y[b,k] = scale * sum_i( silu(x[b,i])*W[i,k]
                              + sum_j basis_j(x[b,i]) * C[i,k,j] )

With a uniform grid (12 knots, spacing h, first knot g0), the cubic
B-spline basis telescopes into truncated relu-cubes:
  y = [r_0..r_10, silu(x)] @ Wfull,   r_s = relu(u-s)^3, u = (x-g0)/h
a [B, 12*128] @ [12*128, 128] matmul.

v2 design (from trace analysis of the 44.3us v1):
- One fused DVE op per shift computes mask+affine+relu-cube directly
  from x (11 passes instead of 12; DVE is the critical engine at
  ~1.13us per [128,1024] pass).
- Mixed-precision matmul: chunks s=0..6 need fp32 (the truncated-power
  basis cancels catastrophically; host-simulated), chunks 7..10 and the
  silu chunk run in fp16 (1 cycle/row vs 4 for fp32) => ~25% less PE
  time. Host-simulated rel err ~6e-3 vs tolerance 2e-2.
- x streamed in as fp16 (halves input DMA; rel err 4e-4), y out as
  fp16 (halves output DMA; 3e-4).
- Matmuls are interleaved with the DVE chunk loop (2 PSUM banks, one
  per 512-column half) so the PE trails the DVE by one chunk instead
  of serializing 12us of matmul tail after the features.

Sharding: data-parallel over batch, 1024 rows per core on 8 cores.
"""

import math
import sys

import numpy as np

if "/opt/trn_rl_repo" not in sys.path:
    sys.path.insert(0, "/opt/trn_rl_repo")

import concourse.bass as bass
import concourse.mybir as mybir
from concourse import bacc
from concourse.tile import TileContext

B_TOTAL = 8192
IN_DIM = 128
OUT_DIM = 128
N_CORES = 8
B_CORE = B_TOTAL // N_CORES  # 1024
NB = 8     # num basis
NS = 11    # truncated-cube shifts s=0..10
K32 = 7    # chunks 0..K32-1 in fp32; rest fp16
N16 = NS - K32 + 1  # fp16 chunks: cubes K32..10 plus the silu chunk

F32 = mybir.dt.float32
F16 = mybir.dt.float16

# ---------------------------------------------------------------- custom DVE op


def _register_ops():
    from concourse.dve_ops import (
        _CUSTOM_DVE_ROW_BASE,
        _SUB_OPCODE_FOR_NAME,
        CUSTOM_DVE_SPECS,
        OPS,
        DveOp,
    )
    from concourse.dve_spec import C0, C1, C2, Spec, Src0, lower, relu, sq
    from concourse.dve_uop import DveOpSpec

    def reg(name, spec):
        for op in OPS:
            if op.name == name:
                return op
        row = _CUSTOM_DVE_ROW_BASE + len(OPS)
        assert row < 0x20
        _SUB_OPCODE_FOR_NAME[name] = row
        shas = {}
        for ver in ("v3", "v4"):
            s = DveOpSpec(name=name, opcode=row, uops=lower(spec, ver=ver),
                          rd1_en=False)
            shas[ver] = s.sha(ver)
        op = DveOp(name, spec, subdim=False, uops_sha=shas)
        OPS.append(op)
        CUSTOM_DVE_SPECS[name] = spec
        return op

    # r = relu(t)^2 * t with t = (x*C0 + C2) * (1 - (x >= C1))
    # i.e. masked, shifted u followed by a truncated cube, all in one pass.
    _t0 = Src0 * C0 + C2
    _t = _t0 - _t0 * (Src0 >= C1)
    cube_mask = Spec(
        body=sq(relu(_t)) * _t,
        reference=lambda in0, in1, s0, s1, imm2: (
            (lambda t: (np.maximum(t, 0.0) ** 2 * t))(
                (in0 * s0 + imm2) * (1.0 - (in0 >= s1))
            )
        ).astype(np.float32),
    )
    return reg("ANT_KAN_CUBE_MASK", cube_mask)


OP_CUBE_MASK = _register_ops()

# ---------------------------------------------------------------- device kernel

_NC_CACHE = {}


def _build_nc():
    if "nc" in _NC_CACHE:
        return _NC_CACHE["nc"]
    inv_h = _NC_CACHE["inv_h"]
    xmax = _NC_CACHE["xmax"]
    u_off = _NC_CACHE["u_off"]

    nc = bacc.Bacc("TRN2", target_bir_lowering=False)
    xT = nc.dram_tensor("xT", [IN_DIM, B_CORE], F16, kind="ExternalInput")
    w32 = nc.dram_tensor("w32", [IN_DIM, K32, OUT_DIM], F32, kind="ExternalInput")
    w16 = nc.dram_tensor("w16", [IN_DIM, N16, OUT_DIM], F16, kind="ExternalInput")
    yT = nc.dram_tensor("yT", [OUT_DIM, B_CORE], F16, kind="ExternalOutput")

    SILU = N16 - 1  # index of the silu chunk within the fp16 chunk group
    HALF = B_CORE // 2

    with TileContext(nc) as tc:
        with (
            tc.tile_pool(name="wpool", bufs=1) as wpool,
            tc.tile_pool(name="dpool", bufs=1) as dpool,
            tc.tile_pool(name="ppool", bufs=2, space="PSUM") as ppool,
        ):
            xt = dpool.tile([IN_DIM, B_CORE], F16, tag="xt")
            nc.sync.dma_start(out=xt[:], in_=xT[:])

            wt16 = wpool.tile([IN_DIM, N16, OUT_DIM], F16, tag="wt16")
            nc.gpsimd.dma_start(out=wt16[:], in_=w16[:])
            wt32 = wpool.tile([IN_DIM, K32, OUT_DIM], F32, tag="wt32")
            nc.gpsimd.dma_start(out=wt32[:], in_=w32[:])

            feat32 = dpool.tile([IN_DIM, K32, B_CORE], F32, tag="feat32")
            feat16 = dpool.tile([IN_DIM, N16, B_CORE], F16, tag="feat16")

            ps = [ppool.tile([OUT_DIM, HALF], F32, name=f"ps{h}", tag=f"ps{h}")
                  for h in (0, 1)]

            # silu chunk on the Activation engine (frees the DVE), fp16 out
            nc.scalar.activation(feat16[:, SILU, :], xt[:],
                                 mybir.ActivationFunctionType.Silu)
            for h in (0, 1):
                nc.tensor.matmul(
                    ps[h][:],
                    lhsT=wt16[:, SILU, :],
                    rhs=feat16[:, SILU, h * HALF:(h + 1) * HALF],
                    start=True,
                    stop=False,
                )

            # truncated relu-cubes, one fused DVE pass per shift
            for s in range(NS):
                if s < K32:
                    f = feat32[:, s, :]
                else:
                    f = feat16[:, s - K32, :]
                nc.vector._custom_dve(
                    OP_CUBE_MASK, out=f, in0=xt[:],
                    s0=inv_h, s1=xmax, imm2=u_off - float(s),
                )
                for h in (0, 1):
                    nc.tensor.matmul(
                        ps[h][:],
                        lhsT=(wt32[:, s, :] if s < K32
                              else wt16[:, s - K32, :]),
                        rhs=(feat32[:, s, h * HALF:(h + 1) * HALF] if s < K32
                             else feat16[:, s - K32, h * HALF:(h + 1) * HALF]),
                        start=False,
                        stop=(s == NS - 1),
                    )

            yt = dpool.tile([OUT_DIM, B_CORE], F16, tag="yt")
            for h in (0, 1):
                nc.scalar.activation(yt[:, h * HALF:(h + 1) * HALF], ps[h][:],
                                     mybir.ActivationFunctionType.Copy)
            nc.sync.dma_start(out=yT[:], in_=yt[:])

    nc.finalize()
    _NC_CACHE["nc"] = nc
    return nc


# ---------------------------------------------------------------- host wrapper


def _build_weights(grid, spline_coeff, base_weight, scale):
    g0 = float(grid[0, 0])
    h = float(grid[0, 1] - grid[0, 0])
    sc = float(np.asarray(scale).reshape(-1)[0])
    # D[j, s]: coefficient of relu(u-s)^3 in B3(u-j), s <= NS-1
    D = np.zeros((NB, NS), dtype=np.float64)
    for j in range(NB):
        for m in range(5):
            s = j + m
            if s < NS:
                D[j, s] = (-1.0) ** m * math.comb(4, m) / 6.0
    C2 = np.einsum("ikj,js->iks", np.asarray(spline_coeff, np.float64), D) * sc
    C2 = C2.transpose(0, 2, 1)  # [i, s, k]
    w32 = np.ascontiguousarray(C2[:, :K32, :]).astype(np.float32)
    w16 = np.empty((IN_DIM, N16, OUT_DIM), dtype=np.float16)
    w16[:, : NS - K32, :] = C2[:, K32:, :]
    w16[:, NS - K32, :] = np.asarray(base_weight, np.float64) * sc
    return w32, w16, g0, h


def _prepare(x, grid, spline_coeff, base_weight, scale):
    w32, w16, g0, h = _build_weights(grid, spline_coeff, base_weight, scale)
    _NC_CACHE.setdefault("inv_h", 1.0 / h)          # 2.5
    _NC_CACHE.setdefault("xmax", g0 + 11.0 * h)     # 2.2 (mask: u < 11)
    _NC_CACHE.setdefault("u_off", -g0 / h)          # 5.5
    nc = _build_nc()

    xT = np.ascontiguousarray(np.asarray(x, np.float32).T.astype(np.float16))
    in_maps = []
    for c in range(N_CORES):
        in_maps.append({
            "xT": np.ascontiguousarray(xT[:, c * B_CORE:(c + 1) * B_CORE]),
            "w32": w32,
            "w16": w16,
        })
    return nc, in_maps


def _postprocess(res):
    yT = np.concatenate(
        [res.results[c]["yT"] for c in range(N_CORES)], axis=1)
    return np.ascontiguousarray(yT.T.astype(np.float32))


def kernel(x, grid, spline_coeff, base_weight, scale):
    from concourse.bass_utils import run_bass_kernel_spmd

    nc, in_maps = _prepare(x, grid, spline_coeff, base_weight, scale)
    res = run_bass_kernel_spmd(nc, in_maps, core_ids=list(range(N_CORES)))
    return _postprocess(res)


if __name__ == "__main__":
    rng = np.random.default_rng(0)
    x = rng.standard_normal((B_TOTAL, IN_DIM)).astype(np.float32)
    g = np.linspace(-1, 1, 6)
    hh = 0.4
    for _ in range(3):
        g = np.concatenate([[g[0] - hh], g, [g[-1] + hh]])
    grid = np.broadcast_to(g.astype(np.float32), (IN_DIM, 12)).copy()
    C = rng.standard_normal((IN_DIM, OUT_DIM, NB)).astype(np.float32)
    W = rng.standard_normal((IN_DIM, OUT_DIM)).astype(np.float32)
    s = np.ones((1,), np.float32)
    y = kernel(x, grid, C, W, s)
    print(y.shape, y.dtype, np.abs(y).max())
